# revision 12
# baseline (speedup 1.0000x reference)
"""Fused BN(inference)+ReLU -> 1x1 conv (512->256) -> 2x2 avgpool on 8 TRN2 cores.

Full inputs in, full output out. Data-parallel over batch (16 -> 2 per core),
BN params + conv weights replicated.

Math folding (host side, tiny):
  s = bn_weight / sqrt(bn_var + eps)            [512]
  t = bn_bias - bn_mean * s                     [512]
  y = relu(s * x + t)                           (one ACT op per channel tile)
  avgpool2x2(W @ y) == (0.25 * W) @ sumpool2x2(y)   (pool before matmul: 4x
                                                     fewer matmul FLOPs)
  wt = 0.25 * W.T                               [512, 256] (lhsT layout)
"""

import copy as _copy

import numpy as np

import bass_rust
import concourse.bass as bass
import concourse.mybir as mybir
import concourse.tile as tile_mod
from concourse.bass_utils import run_bass_kernel_spmd

EPS = 1e-5

B, C_IN, C_OUT, H, W = 16, 512, 256, 56, 56
N_CORES = 8
B_PC = B // N_CORES          # batches per core
HW = H * W                   # 3136
HWP = (H // 2) * (W // 2)    # 784 pooled spatial
K_TILES = C_IN // 128        # 4
M_TILES = C_OUT // 128       # 2
N_CHUNK = HWP // 2           # 392 (fits one PSUM bank)

_DT = mybir.dt.float32


# This walrus build enforces per-instruction sync-wait caps that Tile's
# add_semaphores pass does not respect: CTRL-type instructions (Drain, NoOp)
# take no sem-ge waits at all, EventSemaphore takes at most 2, and every
# other instruction takes at most 1. Post-pass: hoist excess waits onto
# EventSemaphore carrier instructions inserted just before the owning
# instruction on the same engine (same blocking semantics - the carrier
# blocks the engine's sequencer until its waits pass).
_CTRL_OPS = ("InstDrain", "InstNoOp")


def _hoist_excess_waits(nc):
    ev_counter = [0]

    def make_carrier(engine, waits):
        ev_counter[0] += 1
        return mybir.InstEventSemaphore(
            name=f"EVHOIST-{ev_counter[0]}",
            engine=engine,
            ins=[],
            outs=[],
            sync_info=bass_rust.SyncInfo(on_wait=waits, on_update=[]),
        )

    new_module = _copy.replace(nc.m, functions=[])
    for function in nc.m.functions:
        new_function = _copy.replace(function, blocks=[])
        new_function.set_allocations_from_list(function.allocations)
        for block in function.blocks:
            new_insts = []
            for ins in block.instructions:
                si = ins.sync_info
                waits = list(si.on_wait) if si is not None else []
                opname = type(ins).__name__
                if opname in _CTRL_OPS:
                    keep = [w for w in waits if w.wait_mode != "sem-ge-imm"]
                    excess = [w for w in waits if w.wait_mode == "sem-ge-imm"]
                else:
                    limit = 2 if opname == "InstEventSemaphore" else 1
                    keep, excess = waits[:limit], waits[limit:]
                if excess:
                    for i in range(0, len(excess), 2):
                        new_insts.append(make_carrier(ins.engine, excess[i : i + 2]))
                    si.on_wait = keep
                new_insts.append(ins)
            new_function.blocks.append(_copy.replace(block, instructions=new_insts))
        new_module.functions.append(new_function)
    nc.m = new_module


def build_bass():
    nc = bass.Bass()

    # Params come pre-transposed from the host into partition-major layouts so
    # their DMAs are fully contiguous (the naive "(k p) -> p k" gather is 512
    # tiny reads and stalls the HWDGE FIFO ahead of the x stream).
    x_d = nc.dram_tensor("x", [B_PC, C_IN, H, W], _DT, kind="ExternalInput")
    s_d = nc.dram_tensor("s", [128, K_TILES], _DT, kind="ExternalInput")
    t_d = nc.dram_tensor("t", [128, K_TILES], _DT, kind="ExternalInput")
    wt_d = nc.dram_tensor(
        "wt", [128, K_TILES, C_OUT], _DT, kind="ExternalInput"
    )
    out_d = nc.dram_tensor(
        "out", [B_PC, C_OUT, H // 2, W // 2], _DT, kind="ExternalOutput"
    )

    with tile_mod.TileContext(nc) as tc:
        with (
            tc.tile_pool(name="const", bufs=1) as cpool,
            tc.tile_pool(name="xs", bufs=3) as xpool,
            tc.tile_pool(name="ys", bufs=2) as ypool,
            tc.tile_pool(name="us", bufs=2) as upool,
            tc.tile_pool(name="ps", bufs=2 * K_TILES) as ppool,
            tc.tile_pool(name="os", bufs=4) as opool,
            tc.tile_pool(name="psum", bufs=8, space="PSUM") as pspool,
        ):
            # Replicated params, contiguous partition-major DMAs on the ACT
            # HWDGE ring so the x stream owns the SP ring from t=0.
            wt_sb = cpool.tile([128, K_TILES, C_OUT], _DT)
            nc.scalar.dma_start(out=wt_sb[:], in_=wt_d[:])
            s_sb = cpool.tile([128, K_TILES], _DT)
            nc.scalar.dma_start(out=s_sb[:], in_=s_d[:])
            t_sb = cpool.tile([128, K_TILES], _DT)
            nc.scalar.dma_start(out=t_sb[:], in_=t_d[:])

            for b in range(B_PC):
                psums = {}
                for k in range(K_TILES):
                    x_t = xpool.tile([128, HW], _DT)
                    nc.sync.dma_start(
                        out=x_t[:],
                        in_=x_d[b, k * 128 : (k + 1) * 128].rearrange(
                            "c h w -> c (h w)"
                        ),
                    )
                    # y = relu(s*x + t), per-partition scale/bias
                    y_t = ypool.tile([128, HW], _DT)
                    nc.scalar.activation(
                        y_t[:],
                        x_t[:],
                        mybir.ActivationFunctionType.Relu,
                        bias=t_sb[:, k : k + 1],
                        scale=s_sb[:, k : k + 1],
                    )
                    # sum-pool W pairs: u[p, i] = y[p, 2i] + y[p, 2i+1]
                    u_t = upool.tile([128, HW // 2], _DT)
                    yv = y_t[:].rearrange("p (a two) -> p a two", two=2)
                    nc.vector.tensor_add(u_t[:], yv[:, :, 0], yv[:, :, 1])
                    # sum-pool H pairs: p[c, h, w] = u[c, 2h, w] + u[c, 2h+1, w]
                    p_t = ppool.tile([128, HWP], _DT)
                    uv = u_t[:].rearrange(
                        "p (h two w) -> p h two w", two=2, w=W // 2
                    )
                    pv = p_t[:].rearrange("p (h w) -> p h w", w=W // 2)
                    nc.vector.tensor_add(pv, uv[:, :, 0, :], uv[:, :, 1, :])
                    # accumulate this k-slice into all 4 psum banks right away
                    for m in range(M_TILES):
                        for n in range(2):
                            if k == 0:
                                psums[(m, n)] = pspool.tile(
                                    [128, N_CHUNK], _DT, tag="psum",
                                    name=f"psum_{b}_{m}_{n}"
                                )
                            nc.tensor.matmul(
                                psums[(m, n)][:],
                                wt_sb[:, k, m * 128 : (m + 1) * 128],
                                p_t[:, n * N_CHUNK : (n + 1) * N_CHUNK],
                                start=(k == 0),
                                stop=(k == K_TILES - 1),
                            )

                for m in range(M_TILES):
                    o_t = opool.tile([128, HWP], _DT)
                    for n in range(2):
                        # PSUM -> SBUF (DMA can't read PSUM); alternate engines
                        nsl = slice(n * N_CHUNK, (n + 1) * N_CHUNK)
                        if n == 0:
                            nc.scalar.copy(o_t[:, nsl], psums[(m, n)][:])
                        else:
                            nc.vector.tensor_copy(o_t[:, nsl], psums[(m, n)][:])
                    nc.sync.dma_start(
                        out=out_d[b, m * 128 : (m + 1) * 128].rearrange(
                            "o h w -> o (h w)"
                        ),
                        in_=o_t[:],
                    )
    _hoist_excess_waits(nc)
    return nc


_NC_CACHE = None


def _get_nc():
    global _NC_CACHE
    if _NC_CACHE is None:
        _NC_CACHE = build_bass()
    return _NC_CACHE


def _prep_host(bn_weight, bn_bias, bn_mean, bn_var, conv_weight):
    s = (bn_weight / np.sqrt(bn_var + EPS)).astype(np.float32)
    t = (bn_bias - bn_mean * s).astype(np.float32)
    wt = (0.25 * conv_weight.T).astype(np.float32)  # [C_IN, C_OUT]
    # partition-major layouts: [128, K] for vectors, [128, K, C_OUT] for wt
    s2 = np.ascontiguousarray(s.reshape(K_TILES, 128).T)
    t2 = np.ascontiguousarray(t.reshape(K_TILES, 128).T)
    wt2 = np.ascontiguousarray(
        wt.reshape(K_TILES, 128, C_OUT).transpose(1, 0, 2)
    )
    return s2, t2, wt2


def _install_ntff_hook():
    # The agent image's antenv lacks axon_hooks; synthesize it from the boot
    # shim's ctypes factory so trace=True captures NTFF profiles.
    import sys
    import types

    try:
        import antenv.axon_hooks  # noqa: F401

        return
    except ImportError:
        pass
    from trn_agent_boot.trn_boot import _ntff_profile_via_ctypes

    hook = _ntff_profile_via_ctypes("/opt/axon/libaxon_pjrt.so")
    mod = types.ModuleType("antenv.axon_hooks")
    store = {"h": hook}
    mod.get_axon_ntff_profile_hook = lambda: store["h"]
    mod.set_axon_ntff_profile_hook = lambda h: store.__setitem__("h", h)
    import antenv

    antenv.axon_hooks = mod
    sys.modules["antenv.axon_hooks"] = mod


def kernel(x, bn_weight, bn_bias, bn_mean, bn_var, conv_weight, _trace=False):
    if _trace:
        _install_ntff_hook()
    x = np.asarray(x, dtype=np.float32)
    s, t, wt = _prep_host(
        np.asarray(bn_weight, dtype=np.float32),
        np.asarray(bn_bias, dtype=np.float32),
        np.asarray(bn_mean, dtype=np.float32),
        np.asarray(bn_var, dtype=np.float32),
        np.asarray(conv_weight, dtype=np.float32),
    )
    in_maps = [
        {"x": np.ascontiguousarray(x[c * B_PC : (c + 1) * B_PC]), "s": s, "t": t, "wt": wt}
        for c in range(N_CORES)
    ]
    nc = _get_nc()
    res = run_bass_kernel_spmd(
        nc, in_maps, core_ids=list(range(N_CORES)), trace=_trace
    )
    out = np.concatenate([res.results[c]["out"] for c in range(N_CORES)], axis=0)
    if _trace:
        return out, res
    return out


# revision 13
# speedup vs baseline: 1.1194x; 1.1194x over previous
"""Fused BN(inference)+ReLU -> 1x1 conv (512->256) -> 2x2 avgpool on 8 TRN2 cores.

Full inputs in, full output out. Data-parallel over batch (16 -> 2 per core),
BN params + conv weights replicated.

Math folding (host side, tiny):
  s = bn_weight / sqrt(bn_var + eps)            [512]
  t = bn_bias - bn_mean * s                     [512]
  y = relu(s * x + t)                           (one ACT op per channel tile)
  avgpool2x2(W @ y) == (0.25 * W) @ sumpool2x2(y)   (pool before matmul: 4x
                                                     fewer matmul FLOPs)
  wt = 0.25 * W.T                               [512, 256] (lhsT layout)
"""

import copy as _copy

import numpy as np

import bass_rust
import concourse.bass as bass
import concourse.mybir as mybir
import concourse.tile as tile_mod
from concourse.bass_utils import run_bass_kernel_spmd

EPS = 1e-5

B, C_IN, C_OUT, H, W = 16, 512, 256, 56, 56
N_CORES = 8
B_PC = B // N_CORES          # batches per core
HW = H * W                   # 3136
HWP = (H // 2) * (W // 2)    # 784 pooled spatial
K_TILES = C_IN // 128        # 4
M_TILES = C_OUT // 128       # 2
N_CHUNK = HWP // 2           # 392 (fits one PSUM bank)

_DT = mybir.dt.float32


# This walrus build enforces per-instruction sync-wait caps that Tile's
# add_semaphores pass does not respect: CTRL-type instructions (Drain, NoOp)
# take no sem-ge waits at all, EventSemaphore takes at most 2, and every
# other instruction takes at most 1. Post-pass: hoist excess waits onto
# EventSemaphore carrier instructions inserted just before the owning
# instruction on the same engine (same blocking semantics - the carrier
# blocks the engine's sequencer until its waits pass).
_CTRL_OPS = ("InstDrain", "InstNoOp")


def _hoist_excess_waits(nc):
    ev_counter = [0]

    def make_carrier(engine, waits):
        ev_counter[0] += 1
        return mybir.InstEventSemaphore(
            name=f"EVHOIST-{ev_counter[0]}",
            engine=engine,
            ins=[],
            outs=[],
            sync_info=bass_rust.SyncInfo(on_wait=waits, on_update=[]),
        )

    new_module = _copy.replace(nc.m, functions=[])
    for function in nc.m.functions:
        new_function = _copy.replace(function, blocks=[])
        new_function.set_allocations_from_list(function.allocations)
        for block in function.blocks:
            new_insts = []
            for ins in block.instructions:
                si = ins.sync_info
                waits = list(si.on_wait) if si is not None else []
                opname = type(ins).__name__
                if opname in _CTRL_OPS:
                    keep = [w for w in waits if w.wait_mode != "sem-ge-imm"]
                    excess = [w for w in waits if w.wait_mode == "sem-ge-imm"]
                else:
                    limit = 2 if opname == "InstEventSemaphore" else 1
                    keep, excess = waits[:limit], waits[limit:]
                if excess:
                    for i in range(0, len(excess), 2):
                        new_insts.append(make_carrier(ins.engine, excess[i : i + 2]))
                    si.on_wait = keep
                new_insts.append(ins)
            new_function.blocks.append(_copy.replace(block, instructions=new_insts))
        new_module.functions.append(new_function)
    nc.m = new_module


def build_bass():
    nc = bass.Bass()

    # Params come pre-transposed from the host into partition-major layouts so
    # their DMAs are fully contiguous (the naive "(k p) -> p k" gather is 512
    # tiny reads and stalls the HWDGE FIFO ahead of the x stream).
    x_d = nc.dram_tensor("x", [B_PC, C_IN, H, W], _DT, kind="ExternalInput")
    s_d = nc.dram_tensor("s", [128, K_TILES], _DT, kind="ExternalInput")
    t_d = nc.dram_tensor("t", [128, K_TILES], _DT, kind="ExternalInput")
    wt_d = nc.dram_tensor(
        "wt", [128, K_TILES, C_OUT], _DT, kind="ExternalInput"
    )
    out_d = nc.dram_tensor(
        "out", [B_PC, C_OUT, H // 2, W // 2], _DT, kind="ExternalOutput"
    )

    with tile_mod.TileContext(nc) as tc:
        with (
            tc.tile_pool(name="const", bufs=1) as cpool,
            tc.tile_pool(name="xs", bufs=3) as xpool,
            tc.tile_pool(name="ys", bufs=2) as ypool,
            tc.tile_pool(name="us", bufs=2) as upool,
            tc.tile_pool(name="ps", bufs=2 * K_TILES) as ppool,
            tc.tile_pool(name="os", bufs=4) as opool,
            tc.tile_pool(name="psum", bufs=8, space="PSUM") as pspool,
        ):
            # Replicated params, contiguous partition-major DMAs on the ACT
            # HWDGE ring so the x stream owns the SP ring from t=0.
            wt_sb = cpool.tile([128, K_TILES, C_OUT], _DT)
            nc.scalar.dma_start(out=wt_sb[:], in_=wt_d[:])
            s_sb = cpool.tile([128, K_TILES], _DT)
            nc.scalar.dma_start(out=s_sb[:], in_=s_d[:])
            t_sb = cpool.tile([128, K_TILES], _DT)
            nc.scalar.dma_start(out=t_sb[:], in_=t_d[:])

            # Half-spatial chunks: 28 input rows (-> 14 pooled rows = one
            # PSUM n-chunk of 392) per step. Finer granularity shortens the
            # pipeline fill and drain; each chunk feeds its psum bank
            # immediately.
            HC = HW // 2  # 1568 input elems per chunk
            for b in range(B_PC):
                psums = {}
                for k in range(K_TILES):
                    for c in range(2):
                        x_t = xpool.tile([128, HC], _DT)
                        nc.sync.dma_start(
                            out=x_t[:],
                            in_=x_d[
                                b,
                                k * 128 : (k + 1) * 128,
                                c * (H // 2) : (c + 1) * (H // 2),
                            ].rearrange("ch h w -> ch (h w)"),
                        )
                        # y = relu(s*x + t), per-partition scale/bias
                        y_t = ypool.tile([128, HC], _DT)
                        nc.scalar.activation(
                            y_t[:],
                            x_t[:],
                            mybir.ActivationFunctionType.Relu,
                            bias=t_sb[:, k : k + 1],
                            scale=s_sb[:, k : k + 1],
                        )
                        # sum-pool W pairs: u = y[..., 0::2] + y[..., 1::2]
                        u_t = upool.tile([128, HC // 2], _DT)
                        yv = y_t[:].rearrange("p (a two) -> p a two", two=2)
                        nc.vector.tensor_add(u_t[:], yv[:, :, 0], yv[:, :, 1])
                        # sum-pool H pairs
                        p_t = ppool.tile([128, N_CHUNK], _DT)
                        uv = u_t[:].rearrange(
                            "p (h two w) -> p h two w", two=2, w=W // 2
                        )
                        pv = p_t[:].rearrange("p (h w) -> p h w", w=W // 2)
                        nc.vector.tensor_add(pv, uv[:, :, 0, :], uv[:, :, 1, :])
                        # accumulate into both m psum banks for this n-chunk
                        for m in range(M_TILES):
                            if k == 0:
                                psums[(m, c)] = pspool.tile(
                                    [128, N_CHUNK],
                                    _DT,
                                    tag="psum",
                                    name=f"psum_{b}_{m}_{c}",
                                )
                            nc.tensor.matmul(
                                psums[(m, c)][:],
                                wt_sb[:, k, m * 128 : (m + 1) * 128],
                                p_t[:],
                                start=(k == 0),
                                stop=(k == K_TILES - 1),
                            )

                for m in range(M_TILES):
                    o_t = opool.tile([128, HWP], _DT)
                    for n in range(2):
                        # PSUM -> SBUF (DMA can't read PSUM); alternate engines
                        nsl = slice(n * N_CHUNK, (n + 1) * N_CHUNK)
                        if n == 0:
                            nc.scalar.copy(o_t[:, nsl], psums[(m, n)][:])
                        else:
                            nc.vector.tensor_copy(o_t[:, nsl], psums[(m, n)][:])
                    # out on the ACT HWDGE ring: never queues behind x loads
                    nc.scalar.dma_start(
                        out=out_d[b, m * 128 : (m + 1) * 128].rearrange(
                            "o h w -> o (h w)"
                        ),
                        in_=o_t[:],
                    )
    _hoist_excess_waits(nc)
    return nc


_NC_CACHE = None


def _get_nc():
    global _NC_CACHE
    if _NC_CACHE is None:
        _NC_CACHE = build_bass()
    return _NC_CACHE


def _prep_host(bn_weight, bn_bias, bn_mean, bn_var, conv_weight):
    s = (bn_weight / np.sqrt(bn_var + EPS)).astype(np.float32)
    t = (bn_bias - bn_mean * s).astype(np.float32)
    wt = (0.25 * conv_weight.T).astype(np.float32)  # [C_IN, C_OUT]
    # partition-major layouts: [128, K] for vectors, [128, K, C_OUT] for wt
    s2 = np.ascontiguousarray(s.reshape(K_TILES, 128).T)
    t2 = np.ascontiguousarray(t.reshape(K_TILES, 128).T)
    wt2 = np.ascontiguousarray(
        wt.reshape(K_TILES, 128, C_OUT).transpose(1, 0, 2)
    )
    return s2, t2, wt2


def _install_ntff_hook():
    # The agent image's antenv lacks axon_hooks; synthesize it from the boot
    # shim's ctypes factory so trace=True captures NTFF profiles.
    import sys
    import types

    try:
        import antenv.axon_hooks  # noqa: F401

        return
    except ImportError:
        pass
    from trn_agent_boot.trn_boot import _ntff_profile_via_ctypes

    hook = _ntff_profile_via_ctypes("/opt/axon/libaxon_pjrt.so")
    mod = types.ModuleType("antenv.axon_hooks")
    store = {"h": hook}
    mod.get_axon_ntff_profile_hook = lambda: store["h"]
    mod.set_axon_ntff_profile_hook = lambda h: store.__setitem__("h", h)
    import antenv

    antenv.axon_hooks = mod
    sys.modules["antenv.axon_hooks"] = mod


def kernel(x, bn_weight, bn_bias, bn_mean, bn_var, conv_weight, _trace=False):
    if _trace:
        _install_ntff_hook()
    x = np.asarray(x, dtype=np.float32)
    s, t, wt = _prep_host(
        np.asarray(bn_weight, dtype=np.float32),
        np.asarray(bn_bias, dtype=np.float32),
        np.asarray(bn_mean, dtype=np.float32),
        np.asarray(bn_var, dtype=np.float32),
        np.asarray(conv_weight, dtype=np.float32),
    )
    in_maps = [
        {"x": np.ascontiguousarray(x[c * B_PC : (c + 1) * B_PC]), "s": s, "t": t, "wt": wt}
        for c in range(N_CORES)
    ]
    nc = _get_nc()
    res = run_bass_kernel_spmd(
        nc, in_maps, core_ids=list(range(N_CORES)), trace=_trace
    )
    out = np.concatenate([res.results[c]["out"] for c in range(N_CORES)], axis=0)
    if _trace:
        return out, res
    return out


# revision 20
# speedup vs baseline: 1.3179x; 1.1773x over previous
"""Fused BN(inference)+ReLU -> 1x1 conv (512->256) -> 2x2 avgpool on 8 TRN2 cores.

Full inputs in, full output out. Data-parallel over batch (16 -> 2 per core),
BN params + conv weights replicated.

Math folding (host side, tiny):
  s = bn_weight / sqrt(bn_var + eps)            [512]
  t = bn_bias - bn_mean * s                     [512]
  y = relu(s * x + t)                           (one ACT op per channel tile)
  avgpool2x2(W @ y) == (0.25 * W) @ sumpool2x2(y)   (pool before matmul: 4x
                                                     fewer matmul FLOPs)
  wt = 0.25 * W.T                               [512, 256] (lhsT layout)
"""

import copy as _copy

import numpy as np

import bass_rust
import concourse.bass as bass
import concourse.mybir as mybir
import concourse.tile as tile_mod
from concourse.bass_utils import run_bass_kernel_spmd

EPS = 1e-5

B, C_IN, C_OUT, H, W = 16, 512, 256, 56, 56
N_CORES = 8
B_PC = B // N_CORES          # batches per core
HW = H * W                   # 3136
HWP = (H // 2) * (W // 2)    # 784 pooled spatial
K_TILES = C_IN // 128        # 4
M_TILES = C_OUT // 128       # 2
N_CHUNK = HWP // 2           # 392 (fits one PSUM bank)

_DT = mybir.dt.float32


# This walrus build enforces per-instruction sync-wait caps that Tile's
# add_semaphores pass does not respect: CTRL-type instructions (Drain, NoOp)
# take no sem-ge waits at all, EventSemaphore takes at most 2, and every
# other instruction takes at most 1. Post-pass: hoist excess waits onto
# EventSemaphore carrier instructions inserted just before the owning
# instruction on the same engine (same blocking semantics - the carrier
# blocks the engine's sequencer until its waits pass).
_CTRL_OPS = ("InstDrain", "InstNoOp")


def _hoist_excess_waits(nc):
    ev_counter = [0]

    def make_carrier(engine, waits):
        ev_counter[0] += 1
        return mybir.InstEventSemaphore(
            name=f"EVHOIST-{ev_counter[0]}",
            engine=engine,
            ins=[],
            outs=[],
            sync_info=bass_rust.SyncInfo(on_wait=waits, on_update=[]),
        )

    new_module = _copy.replace(nc.m, functions=[])
    for function in nc.m.functions:
        new_function = _copy.replace(function, blocks=[])
        new_function.set_allocations_from_list(function.allocations)
        for block in function.blocks:
            new_insts = []
            for ins in block.instructions:
                si = ins.sync_info
                waits = list(si.on_wait) if si is not None else []
                opname = type(ins).__name__
                if opname in _CTRL_OPS:
                    keep = [w for w in waits if w.wait_mode != "sem-ge-imm"]
                    excess = [w for w in waits if w.wait_mode == "sem-ge-imm"]
                else:
                    limit = 2 if opname == "InstEventSemaphore" else 1
                    keep, excess = waits[:limit], waits[limit:]
                if excess:
                    for i in range(0, len(excess), 2):
                        new_insts.append(make_carrier(ins.engine, excess[i : i + 2]))
                    si.on_wait = keep
                new_insts.append(ins)
            new_function.blocks.append(_copy.replace(block, instructions=new_insts))
        new_module.functions.append(new_function)
    nc.m = new_module


def build_bass():
    nc = bass.Bass()

    # Params come pre-transposed from the host into partition-major layouts so
    # their DMAs are fully contiguous (the naive "(k p) -> p k" gather is 512
    # tiny reads and stalls the HWDGE FIFO ahead of the x stream).
    x_d = nc.dram_tensor("x", [B_PC, C_IN, H, W], _DT, kind="ExternalInput")
    s_d = nc.dram_tensor("s", [128, K_TILES], _DT, kind="ExternalInput")
    t_d = nc.dram_tensor("t", [128, K_TILES], _DT, kind="ExternalInput")
    wt_d = nc.dram_tensor(
        "wt", [128, K_TILES, C_OUT], _DT, kind="ExternalInput"
    )
    out_d = nc.dram_tensor(
        "out", [B_PC, C_OUT, H // 2, W // 2], _DT, kind="ExternalOutput"
    )

    with tile_mod.TileContext(nc) as tc:
        with (
            tc.tile_pool(name="const", bufs=1) as cpool,
            tc.tile_pool(name="xs", bufs=6) as xpool,
            tc.tile_pool(name="ys", bufs=4) as ypool,
            tc.tile_pool(name="us", bufs=3) as upool,
            tc.tile_pool(name="ps", bufs=3) as ppool,
            tc.tile_pool(name="os", bufs=4) as opool,
            tc.tile_pool(name="psum", bufs=8, space="PSUM") as pspool,
        ):
            # Replicated params, contiguous partition-major DMAs on the ACT
            # HWDGE ring so the x stream owns the SP ring from t=0.
            wt_sb = cpool.tile([128, K_TILES, C_OUT], _DT)
            nc.scalar.dma_start(out=wt_sb[:], in_=wt_d[:])
            s_sb = cpool.tile([128, K_TILES], _DT)
            nc.scalar.dma_start(out=s_sb[:], in_=s_d[:])
            t_sb = cpool.tile([128, K_TILES], _DT)
            nc.scalar.dma_start(out=t_sb[:], in_=t_d[:])

            def emit_chunk(b, k, c, ncols, psums):
                """Process input rows [c*nrows, (c+1)*nrows) of k-slice k:
                DMA -> BN+ReLU -> 2x2 sum-pool -> matmul into psum n-chunks.

                ncols: number of pooled n-chunks this chunk covers (2 = full
                k-tile, 1 = half). c is the chunk index at that granularity.
                """
                nrows = ncols * (H // 2)  # input rows (56 or 28)
                hc = nrows * W
                x_t = xpool.tile([128, hc], _DT, tag="x", name=f"x_{b}_{k}_{c}")
                nc.sync.dma_start(
                    out=x_t[:],
                    in_=x_d[
                        b,
                        k * 128 : (k + 1) * 128,
                        c * nrows : (c + 1) * nrows,
                    ].rearrange("ch h w -> ch (h w)"),
                )
                y_t = ypool.tile([128, hc], _DT, tag="y", name=f"y_{b}_{k}_{c}")
                nc.scalar.activation(
                    y_t[:],
                    x_t[:],
                    mybir.ActivationFunctionType.Relu,
                    bias=t_sb[:, k : k + 1],
                    scale=s_sb[:, k : k + 1],
                )
                u_t = upool.tile(
                    [128, hc // 2], _DT, tag="u", name=f"u_{b}_{k}_{c}"
                )
                yv = y_t[:].rearrange("p (a two) -> p a two", two=2)
                nc.vector.tensor_add(u_t[:], yv[:, :, 0], yv[:, :, 1])
                p_t = ppool.tile(
                    [128, hc // 4], _DT, tag="p", name=f"p_{b}_{k}_{c}"
                )
                uv = u_t[:].rearrange(
                    "p (h two w) -> p h two w", two=2, w=W // 2
                )
                pv = p_t[:].rearrange("p (h w) -> p h w", w=W // 2)
                nc.vector.tensor_add(pv, uv[:, :, 0, :], uv[:, :, 1, :])
                for m in range(M_TILES):
                    for j in range(ncols):
                        n = c * ncols + j
                        if k == 0:
                            psums[(m, n)] = pspool.tile(
                                [128, N_CHUNK],
                                _DT,
                                tag="psum",
                                name=f"psum_{b}_{m}_{n}",
                            )
                        nc.tensor.matmul(
                            psums[(m, n)][:],
                            wt_sb[:, k, m * 128 : (m + 1) * 128],
                            p_t[:, j * N_CHUNK : (j + 1) * N_CHUNK],
                            start=(k == 0),
                            stop=(k == K_TILES - 1),
                        )

            all_psums = []
            for b in range(B_PC):
                psums = {}
                all_psums.append(psums)
                for k in range(K_TILES):
                    first = b == 0 and k == 0
                    last = b == B_PC - 1 and k == K_TILES - 1
                    if first or last:
                        # halve the pipeline-fill/drain chunks
                        emit_chunk(b, k, 0, 1, psums)
                        emit_chunk(b, k, 1, 1, psums)
                    else:
                        emit_chunk(b, k, 0, 2, psums)

                for m in range(M_TILES):
                    o_t = opool.tile([128, HWP], _DT)
                    for n in range(2):
                        # PSUM -> SBUF (DMA can't read PSUM); alternate engines
                        nsl = slice(n * N_CHUNK, (n + 1) * N_CHUNK)
                        if n == 0:
                            nc.scalar.copy(o_t[:, nsl], psums[(m, n)][:])
                        else:
                            nc.vector.tensor_copy(o_t[:, nsl], psums[(m, n)][:])
                    # out on the ACT HWDGE ring: never queues behind x loads
                    nc.scalar.dma_start(
                        out=out_d[b, m * 128 : (m + 1) * 128].rearrange(
                            "o h w -> o (h w)"
                        ),
                        in_=o_t[:],
                    )
    _hoist_excess_waits(nc)
    return nc


_NC_CACHE = None


def _get_nc():
    global _NC_CACHE
    if _NC_CACHE is None:
        _NC_CACHE = build_bass()
    return _NC_CACHE


def _prep_host(bn_weight, bn_bias, bn_mean, bn_var, conv_weight):
    s = (bn_weight / np.sqrt(bn_var + EPS)).astype(np.float32)
    t = (bn_bias - bn_mean * s).astype(np.float32)
    wt = (0.25 * conv_weight.T).astype(np.float32)  # [C_IN, C_OUT]
    # partition-major layouts: [128, K] for vectors, [128, K, C_OUT] for wt
    s2 = np.ascontiguousarray(s.reshape(K_TILES, 128).T)
    t2 = np.ascontiguousarray(t.reshape(K_TILES, 128).T)
    wt2 = np.ascontiguousarray(
        wt.reshape(K_TILES, 128, C_OUT).transpose(1, 0, 2)
    )
    return s2, t2, wt2


def _install_ntff_hook():
    # The agent image's antenv lacks axon_hooks; synthesize it from the boot
    # shim's ctypes factory so trace=True captures NTFF profiles.
    import sys
    import types

    try:
        import antenv.axon_hooks  # noqa: F401

        return
    except ImportError:
        pass
    from trn_agent_boot.trn_boot import _ntff_profile_via_ctypes

    hook = _ntff_profile_via_ctypes("/opt/axon/libaxon_pjrt.so")
    mod = types.ModuleType("antenv.axon_hooks")
    store = {"h": hook}
    mod.get_axon_ntff_profile_hook = lambda: store["h"]
    mod.set_axon_ntff_profile_hook = lambda h: store.__setitem__("h", h)
    import antenv

    antenv.axon_hooks = mod
    sys.modules["antenv.axon_hooks"] = mod


def kernel(x, bn_weight, bn_bias, bn_mean, bn_var, conv_weight, _trace=False):
    if _trace:
        _install_ntff_hook()
    x = np.asarray(x, dtype=np.float32)
    s, t, wt = _prep_host(
        np.asarray(bn_weight, dtype=np.float32),
        np.asarray(bn_bias, dtype=np.float32),
        np.asarray(bn_mean, dtype=np.float32),
        np.asarray(bn_var, dtype=np.float32),
        np.asarray(conv_weight, dtype=np.float32),
    )
    in_maps = [
        {"x": np.ascontiguousarray(x[c * B_PC : (c + 1) * B_PC]), "s": s, "t": t, "wt": wt}
        for c in range(N_CORES)
    ]
    nc = _get_nc()
    res = run_bass_kernel_spmd(
        nc, in_maps, core_ids=list(range(N_CORES)), trace=_trace
    )
    out = np.concatenate([res.results[c]["out"] for c in range(N_CORES)], axis=0)
    if _trace:
        return out, res
    return out
